# revision 60
# baseline (speedup 1.0000x reference)
"""Trainium2 Bass kernel for nn_NoiseProjector.

Strategy (8 NeuronCores):
- Data-parallel conv trunk: each core runs conv1+conv2+GAP on 8 of 64 images.
- Tiny AllGather of the pooled features (8x64 -> 64x64 per core).
- Tensor-parallel FC stage: weights column-sharded over output_dim (128
  outputs per core).
- The third-order contraction  sum_{ijk} fc_i fc_j fc_k W3[o,ijk]  depends
  only on the symmetrization of W3 over (i,j,k).  Host folds all 6
  permutations into a canonical i<=j<=k tensor W''[o,i,(j<=k)] (zero for
  j<i), so the device streams only the block-triangular part:
    T1[b,o,i] = sum_{p=(j<=k), j>=i} covUT[b,p] * W''[o,i,p]
    third[b,o] = sum_i fc[b,i] * T1[b,o,i]
  where covUT[b, p] = fc_j fc_k over the 2080 upper-triangle pairs.
  i is processed in 16 groups of 4 with K-ranges rounded down to the global
  128-chunk grid of covUT^T; weight blob is 13184 rows x 512 cols fp8e4m3
  = 6.75 MB/core (vs 33.5 MB unfolded fp8), with 512B DMA lines.
- cov/mean fc: wc is folded the same way over its symmetric (j,k) -> 2080
  rows f16; wm dense f32.  Biases summed host-side layout unchanged.
- The conv1 input is pre-tiled on the host into the exact (dy,ci)-replicated
  SBUF layout (one contiguous DMA per y-chunk instead of 336 small strided
  DMAs, which cost ~0.7 us each).
- Host concatenates the 8 per-core (64,128) outputs into (64,1024).
"""

import sys

sys.path.insert(0, "/opt/trn_rl_repo")

import numpy as np
import ml_dtypes

B = 64          # global batch
BL = 8          # images per core
NCORES = 8
OPC = 128       # outputs per core
FEAT = 64
JK = 4096       # FEAT*FEAT contraction
NP = 2080       # upper-triangle (j<=k) pairs
NPC = 17        # ceil(NP/128) 128-chunks of the UT pair space
NPPAD = NPC * 128            # 2176, zero-padded pair rows for wc
OFFJ = [64 * j - j * (j - 1) // 2 for j in range(65)]   # UT row offset per j
GRP = 4         # i's per group
NG = 16         # i groups
KCS = [OFFJ[GRP * g] // 128 for g in range(NG)]  # start chunk per group
TOTROWS = sum(128 * (16 - k) + 32 for k in KCS)  # 13184 blob rows
NCH = sum(NPC - k for k in KCS)                  # 115 weight chunks
CSCALE = 512.0  # covUT scaled by this before fp8 cast (wc/stage2 compensate)
H, W = 224, 224
H1, W1 = 112, 112   # conv1 out
H2, W2 = 56, 56     # conv2 out
YC_HOST = 16        # conv1 y-rows per chunk (host pre-tiled layout)
GAP = 1.0 / (H2 * W2)



def _split_multiwait_json(raw):
    """This walrus build accepts only ONE sync wait per instruction.  Split any
    multi-wait instruction into single-wait EventSemaphore ops ahead of it (the
    engine is in-order, so chained waits are equivalent)."""
    import json

    j = json.loads(raw)
    n_split = 0
    for f in j["functions"]:
        for bb in f["blocks"]:
            insts = bb.get("instructions")
            if not insts:
                continue
            out = []
            changed = False
            for ins in insts:
                si = ins.get("sync_info")
                waits = si.get("on_wait") if si else None
                if waits and len(waits) > 1:
                    changed = True
                    keep = None
                    for w in waits:
                        if w.get("wait_reg") is not None:
                            keep = w
                    if keep is None:
                        keep = waits[-1]
                    rest = [w for w in waits if w is not keep]
                    for k, w in enumerate(rest):
                        n_split += 1
                        out.append({
                            "engine": ins["engine"], "ins": [], "outs": [],
                            "name": f"{ins['name']}-sw{k}",
                            "opcode": "EventSemaphore",
                            "sync_info": {"on_update": [], "on_wait": [w]},
                        })
                    si["on_wait"] = [keep]
                out.append(ins)
            if changed:
                bb["instructions"] = out
    return json.dumps(j).encode(), n_split


def _build(reps=1, trivial=False, w3_fp8=None, conv_reps=1, conv1_only=False,
           conv2_seq=False, coll_reps=1):
    YC = YC_HOST
    NQ = H1 // YC
    import concourse.bass as bass
    import concourse.mybir as mybir
    import concourse.tile as tile
    from concourse.masks import make_identity

    F32, F16, BF16 = mybir.dt.float32, mybir.dt.float16, mybir.dt.bfloat16
    AF = mybir.ActivationFunctionType
    ALU = mybir.AluOpType
    AX = mybir.AxisListType



    nc = bass.Bass("TRN2", target_bir_lowering=False, num_devices=NCORES)

    F8 = mybir.dt.float8e4
    DRm = mybir.MatmulPerfMode.DoubleRow
    xb3 = nc.dram_tensor(
        "xb3", (H1 // YC_HOST, 108, 2, YC_HOST, W1), F8,
        kind="ExternalInput").ap()
    w1t = nc.dram_tensor("w1t", (108, 128), F8, kind="ExternalInput").ap()
    b1 = nc.dram_tensor("b1", (32, 1), F32, kind="ExternalInput").ap()
    w2t = nc.dram_tensor("w2t", (128, 3, 3, 128), F8, kind="ExternalInput").ap()
    b2 = nc.dram_tensor("b2", (64, 1), F32, kind="ExternalInput").ap()
    wmt = nc.dram_tensor("wmt", (64, OPC), F32, kind="ExternalInput").ap()
    w3dt = F8
    w3g = nc.dram_tensor("w3g", (128, NCH, 512), w3dt, kind="ExternalInput").ap()
    wct = nc.dram_tensor("wct", (NPPAD, OPC), F16, kind="ExternalInput").ap()
    bias3 = nc.dram_tensor("bias3", (3, OPC), F32, kind="ExternalInput").ap()
    out = nc.dram_tensor("out", (B, OPC), F32, kind="ExternalOutput").ap()
    feat_loc = nc.dram_tensor("feat_loc", (BL, FEAT), F32).ap()
    feat_all = nc.dram_tensor("feat_all", (B, FEAT), F32, addr_space="Shared").ap()

    if trivial:
        with tile.TileContext(nc) as tc:
            with tc.tile_pool(name="tp", bufs=1) as tp:
                z = tp.tile([B, OPC], F32)
                nc.vector.memset(z[:], 0.0)
                nc.sync.dma_start(out[:], z[:])
        nc.finalize()
        fixed, _ = _split_multiwait_json(nc.to_json_bytes())
        nc.to_json_bytes = lambda: fixed
        return nc

    with tile.TileContext(nc) as tc:
        with (
            tc.tile_pool(name="consts", bufs=1) as consts,
            tc.tile_pool(name="fcsingle", bufs=1) as fcsingle,
            tc.tile_pool(name="fcwork", bufs=2) as fcwork,
        ):
            # ---- constants ----
            # block-diagonal conv1 weights: rows 27c+(dy,ci,dx), cols 32c+co
            w1sb = consts.tile([108, 128], F8)
            nc.sync.dma_start(w1sb[:], w1t[:])
            # block-diagonal conv2 weights (2-img blocks), duplicated to both
            # partition halves (stationary must sit in its own row quadrant)
            w2sb = consts.tile([128, 3, 3, 128], F8)
            nc.sync.dma_start(w2sb[:], w2t[:])
            bias1 = consts.tile([128, 1], F32)          # b1[cout] at 32c+cout
            nc.sync.dma_start(
                bias1[:],
                bass.AP(tensor=b1.tensor, offset=0, ap=[[0, 4], [1, 32], [1, 1]]),
            )
            bias2 = consts.tile([128, 1], F32)          # 128x b2 at 64h+co
            nc.sync.dma_start(
                bias2[:],
                bass.AP(tensor=b2.tensor, offset=0, ap=[[0, 2], [1, 64], [1, 1]]),
            )
            wmsb = consts.tile([64, OPC], F32)
            nc.sync.dma_start(wmsb[:], wmt[:])
            wcsb = consts.tile([128, NPC, OPC], F16)    # [p, kc, m] of folded wct
            nc.sync.dma_start(
                wcsb[:], wct[:].rearrange("(kc p) m -> p kc m", p=128)
            )
            bias3sb = consts.tile([64, 3, OPC], F32)
            nc.sync.dma_start(
                bias3sb[:],
                bass.AP(tensor=bias3.tensor, offset=0,
                        ap=[[0, 64], [OPC, 3], [1, OPC]]),
            )
            bsum = consts.tile([64, OPC], F32)
            nc.vector.tensor_reduce(bsum[:], bias3sb[:].transpose([0, 2, 1]),
                                    AX.X, op=ALU.add)
            ident = consts.tile([64, 64], F32)
            make_identity(nc, ident[:])

            # persistent w3 buffer in 8 group-aligned segments; prefetch DMAs
            # issued on the gpsimd queue (own DGE ring) right at kernel start
            # so the whole 7.5MB stream overlaps the conv trunk.  Segmenting
            # lets measurement reps overlap the re-stream with compute.
            GCNT = [NPC - k for k in KCS]            # chunks per group
            SEGG = [[0], [1], [2], [3], [4, 5], [6, 7], [8, 9, 10],
                    [11, 12, 13, 14, 15]]
            w3segs = []
            SEGOF = {}
            for si, gl in enumerate(SEGG):
                n = sum(GCNT[g] for g in gl)
                w3segs.append(consts.tile([128, n, 512], w3dt,
                                          name=f"w3seg{si}"))
                base = sum(GCNT[g] for g in range(gl[0]))
                for lo in range(n):
                    SEGOF[base + lo] = (si, lo)

            def _w3_load():
                c0 = 0
                for si, gl in enumerate(SEGG):
                    n = sum(GCNT[g] for g in gl)
                    nc.gpsimd.dma_start(w3segs[si][:], w3g[:, c0:c0 + n, :])
                    c0 += n

            def _w3slot(ci, nt):
                si, lo = SEGOF[ci]
                if nt == 2:
                    return w3segs[si][:, lo:lo + 2, :]
                return w3segs[si][:, lo, :]

            _w3_load()
            zeros = consts.tile([128, 448], BF16)
            nc.vector.memset(zeros[:], 0.0)
            featparts = consts.tile([128, 2, 2, 7], F32)   # [64h+co, p2, r, sc]
            if conv1_only:
                nc.vector.memset(featparts[:], 0.0)

            # =============== conv trunk (8 local images) ===============
            # conv1: K=(dy,ci)=9 at row groups {0,32}; 4 images per col group.
            # image assignment: img = 2*c + r  (c: col group, r: row group/bank)
            with (
                tc.tile_pool(name="conv", bufs=2) as conv,
                tc.tile_pool(name="h1p", bufs=1) as h1p,
            ):
                # h1 stored fp8, 16x scaled (x*2 * w1*8)
                h1 = h1p.tile([128, 2, H1 + 2, W1 + 2], F8)  # [(32c)+co, r, y+1, x+1]
                nc.vector.memset(h1[:, :, 0:1, :], 0.0)        # top pad row
                nc.vector.memset(h1[:, :, :, 0:1], 0.0)        # left pad col

                def _win(slc, dims):
                    # overlapping-window AP: keep the partition entry of a
                    # sliced AP, replace the free dims
                    return bass.AP(tensor=slc.tensor, offset=slc.offset,
                                   ap=[list(slc.ap[0])] + dims)

                for _crep in range(conv_reps):
                    cpsum_cm = tc.tile_pool(name="cpsum", bufs=3, space="PSUM")
                    cpsum = cpsum_cm.__enter__()
                    for q in range(NQ):
                        a1 = conv.tile([108, 2, YC, W1], F8, tag="a1")
                        nc.sync.dma_start(a1[:], xb3[q])
                        for s in range(YC // 4):
                            ps1 = cpsum.tile([128, 2, 512], F32, tag="cpsum")
                            for rg in range(2):
                                rhs = a1[:, rg, 4 * s:4 * s + 4, :]
                                nc.tensor.matmul(
                                    ps1[:, rg, 0:448],
                                    w1sb[:],
                                    rhs,
                                    start=True, stop=True,
                                )
                            ybase = 1 + q * YC + 4 * s
                            for r in range(2):
                                src = ps1[:, r, 0:448].rearrange("p (y x) -> p y x", y=4)
                                dst = h1[:, r, ybase:ybase + 4, 1:113]
                                if r == 0:
                                    nc.scalar.activation(dst, src, AF.Relu,
                                                         bias=bias1[:], scale=1.0)
                                else:
                                    nc.vector.scalar_tensor_tensor(
                                        dst, src, bias1[:],
                                        zeros[:].rearrange("p (y x) -> p y x", y=4),
                                        op0=ALU.add, op1=ALU.max,
                                    )

                    cpsum_cm.__exit__(None, None, None)

                    if conv1_only:
                        continue
                    # conv2: K=ci=32 at row groups {0,32,64,96} (4 images per
                    # r), M=64; fp8 DoubleRow pairs (dy0,dy1)x dx, (dy2:dx0,dx1),
                    # plus plain (dy2,dx2); 2 y-halves of 224 cols each.
                    with tc.tile_pool(name="c2psum", bufs=2, space="PSUM") as c2psum:
                        trash = consts.tile([128, 448], BF16)
                        trash2 = consts.tile([128, 448], BF16)
                        for r in range(2):
                            for sc in range(7):
                                ps2 = c2psum.tile([128, 2, 512], F32, tag="c2psum")
                                for dy in range(3):
                                    for dx in range(3):
                                        for p2 in range(2):
                                            rhs = h1[64 * p2:64 * p2 + 64, r,
                                                     16 * sc + dy:16 * sc + dy + 16:2,
                                                     dx:dx + 2 * W2:2]
                                            nc.tensor.matmul(
                                                ps2[:, p2, 0:448],
                                                w2sb[64 * p2:64 * p2 + 64, dy, dx, :],
                                                rhs,
                                                start=(dy == 0 and dx == 0),
                                                stop=(dy == 2 and dx == 2),
                                                tile_position=(64 * p2, 0),
                                                skip_group_check=True,
                                            )
                                for p2 in range(2):
                                    for h in range(2):
                                        src = ps2[64 * h:64 * h + 64, p2, 0:448]
                                        fslot = featparts[64 * h:64 * h + 64,
                                                          p2, r, sc:sc + 1]
                                        if (p2 + h) % 2 == 0:
                                            nc.scalar.activation(
                                                trash[64 * h:64 * h + 64, :], src,
                                                AF.Relu,
                                                bias=bias2[64 * h:64 * h + 64],
                                                scale=1.0, accum_out=fslot,
                                            )
                                        else:
                                            nc.vector.scalar_tensor_tensor(
                                                trash2[64 * h:64 * h + 64, :], src,
                                                bias2[64 * h:64 * h + 64],
                                                zeros[64 * h:64 * h + 64, :],
                                                op0=ALU.add, op1=ALU.max,
                                                accum_out=fslot,
                                            )

            # featparts[64h+co, p2, r, sc] holds img 4*p2+2*h+r; move the h=1
            # half down to partitions 0-63 and reduce over sc into featTl
            fpfix = fcsingle.tile([64, 2, 2, 7], F32, tag="fpfix")
            nc.sync.dma_start(fpfix[:], featparts[64:128, :, :, :])
            featTl = fcsingle.tile([64, BL], F32, tag="featTl")
            for p2 in range(2):
                for r in range(2):
                    nc.vector.tensor_reduce(
                        featTl[:, 4 * p2 + r:4 * p2 + r + 1],
                        featparts[0:64, p2, r, :], AX.X, op=ALU.add)
                    nc.vector.tensor_reduce(
                        featTl[:, 4 * p2 + 2 + r:4 * p2 + 3 + r],
                        fpfix[:, p2, r, :], AX.X, op=ALU.add)
            nc.vector.tensor_scalar_mul(featTl[:], featTl[:], GAP / 128.0)
            nc.sync.dma_start(feat_loc[:].transpose([1, 0]), featTl[:])

            for _cl in range(coll_reps):
                nc.gpsimd.collective_compute(
                    "AllGather", ALU.bypass,
                    replica_groups=[list(range(NCORES))],
                    ins=[feat_loc[:]], outs=[feat_all[:]],
                )

            for _rep in range(reps):
                # =============== fc prep ===============
                covT = fcsingle.tile([128, NPC, 64], F8, tag="covT")
                featT = fcsingle.tile([64, 64], F32, tag="featT")
                thirdparts = fcsingle.tile([64, OPC, NG], F32, tag="thirdparts")
                feat = fcsingle.tile([64, 64], F32, tag="feat")
                nc.sync.dma_start(feat[:], feat_all[:])
                mean = fcsingle.tile([64, 1], F32, tag="mean")
                nc.vector.tensor_reduce(mean[:], feat[:], AX.X, op=ALU.add)
                nc.vector.tensor_scalar_mul(mean[:], mean[:], 1.0 / FEAT)
                fc = fcsingle.tile([64, 64], F32, tag="fc")
                nc.vector.tensor_scalar_sub(fc[:], feat[:], mean[:])
                fcS = fcsingle.tile([64, 64], F32, tag="fcS")
                nc.vector.tensor_scalar_mul(fcS[:], fc[:], CSCALE)
                fcI = fcsingle.tile([64, 64], F32, tag="fcI")
                nc.vector.tensor_scalar_mul(fcI[:], fc[:], 1.0 / CSCALE)
                # covUT[b, p] = CSCALE * fc_j * fc_k over UT pairs (j<=k),
                # j-major; construction split across vector and scalar engines
                covUT = fcsingle.tile([64, NP], F32, tag="covUT")
                for j in range(64):
                    if j % 2 == 0:
                        nc.vector.tensor_scalar_mul(
                            covUT[:, OFFJ[j]:OFFJ[j + 1]], fc[:, j:64],
                            fcS[:, j:j + 1])
                    else:
                        nc.scalar.mul(
                            covUT[:, OFFJ[j]:OFFJ[j + 1]], fc[:, j:64],
                            fcS[:, j:j + 1])

                with tc.tile_pool(name="fpsum", bufs=2, space="PSUM") as fpsum, \
                     tc.tile_pool(name="wpsum", bufs=1, space="PSUM") as wpsum:
                    # transposes: covUT -> covT chunks; feat -> featT
                    for kc in range(NPC):
                        cw = min(128, NP - 128 * kc)
                        pT = fpsum.tile([128, 64], F32, tag="pT")
                        nc.tensor.transpose(
                            pT[0:cw, :], covUT[:, 128 * kc:128 * kc + cw], ident[:])
                        if kc % 2 == 0:
                            nc.vector.tensor_copy(covT[0:cw, kc, :], pT[0:cw, :])
                        else:
                            nc.scalar.copy(covT[0:cw, kc, :], pT[0:cw, :])
                    pT2 = fpsum.tile([128, 64], F32, tag="pT")
                    nc.tensor.transpose(pT2[0:64, :], feat[:], ident[:])
                    nc.vector.tensor_copy(featT[:], pT2[0:64, :])

                    # wc + wm accumulation -> psum_wc [64, 128]
                    pwc = wpsum.tile([64, OPC], F32)
                    for kc in range(NPC):
                        cw = min(128, NP - 128 * kc)
                        nc.tensor.matmul(pwc[:], covT[0:cw, kc, :], wcsb[0:cw, kc, :],
                                         start=(kc == 0), stop=False)
                    nc.tensor.matmul(pwc[:], featT[:], wmsb[:], start=False, stop=True)

                    # ---- w3 stream: 16 i-groups, block-triangular K ranges ----
                    if _rep > 0:
                        _w3_load()   # measurement reps re-stream the weights
                    DR = mybir.MatmulPerfMode.DoubleRow
                    ci = 0
                    for g in range(NG):
                        pba = fpsum.tile([64, 256], F32, tag="pba")
                        pbb = fpsum.tile([64, 256], F32, tag="pbb")
                        kcl = list(range(KCS[g], NPC))
                        # pair full chunks for DoubleRow fp8 (2 k-tiles/pass),
                        # then odd full chunk, then the 32-row tail (kc==16)
                        items = []
                        idx = 0
                        while idx + 2 < len(kcl):
                            items.append((2, kcl[idx], ci))
                            idx += 2
                            ci += 2
                        while idx < len(kcl):
                            items.append((1, kcl[idx], ci))
                            idx += 1
                            ci += 1
                        for it, (nt, kc, c0) in enumerate(items):
                            st, sp = (it == 0), (it == len(items) - 1)
                            rows = min(128, NP - 128 * kc)
                            wsl = _w3slot(c0, nt)
                            if nt == 2:
                                lhs = covT[:, kc:kc + 2, :]
                                r0 = wsl[:, :, 0:256]
                                r1 = wsl[:, :, 256:512]
                                pm = DR
                            else:
                                lhs = covT[0:rows, kc, :]
                                r0 = wsl[0:rows, 0:256]
                                r1 = wsl[0:rows, 256:512]
                                pm = None
                            nc.tensor.matmul(pba[:], lhs, r0,
                                             start=st, stop=sp, perf_mode=pm,
                                             tile_position=(0, 0),
                                             skip_group_check=True)
                            nc.tensor.matmul(pbb[:], lhs, r1,
                                             start=st, stop=sp, perf_mode=pm,
                                             tile_position=(0, 0),
                                             skip_group_check=True)
                        # stage 2: third[b, o] += sum_il fc[b, 4g+il] * T1[b, o, il]
                        # a-half: vector mul+reduce; b-half: scalar copies psum
                        # out, gpsimd multiplies, vector reduces (f16 tmps)
                        fcg = fcI[:, GRP * g:GRP * (g + 1)]
                        tmpa = fcwork.tile([64, 256], F16, tag="tmpa")
                        nc.vector.tensor_mul(
                            tmpa[:].rearrange("p (o i) -> p o i", o=64),
                            pba[:].rearrange("p (o i) -> p o i", o=64),
                            fcg.unsqueeze(1).broadcast_to([64, 64, GRP]),
                        )
                        nc.vector.tensor_reduce(
                            thirdparts[:, 0:64, g],
                            tmpa[:].rearrange("p (o i) -> p o i", o=64),
                            AX.X, op=ALU.add,
                        )
                        tmpb = fcwork.tile([64, 256], F16, tag="tmpb")
                        nc.vector.tensor_mul(
                            tmpb[:].rearrange("p (o i) -> p o i", o=64),
                            pbb[:].rearrange("p (o i) -> p o i", o=64),
                            fcg.unsqueeze(1).broadcast_to([64, 64, GRP]),
                        )
                        nc.vector.tensor_reduce(
                            thirdparts[:, 64:128, g],
                            tmpb[:].rearrange("p (o i) -> p o i", o=64),
                            AX.X, op=ALU.add,
                        )

                    # ---- final assembly: out = third + (cov_feat+mean_feat) + biases
                    acc3 = fcsingle.tile([64, OPC], F32, tag="acc3")
                    nc.vector.tensor_reduce(acc3[:], thirdparts[:], AX.X, op=ALU.add)
                    acc = fcsingle.tile([64, OPC], F32, tag="acc")
                    nc.vector.tensor_add(acc[:], acc3[:], pwc[:])
                    nc.vector.tensor_add(acc[:], acc[:], bsum[:])
                    nc.sync.dma_start(out[:], acc[:])

    nc.finalize()
    fixed, n_split = _split_multiwait_json(nc.to_json_bytes())
    nc.to_json_bytes = lambda: fixed
    return nc


_NC_CACHE = None


def _get_nc():
    global _NC_CACHE
    if _NC_CACHE is None:
        _NC_CACHE = _build()
    return _NC_CACHE


def _prepare_in_maps(inputs):
    x = np.asarray(inputs["x"])
    w1 = np.asarray(inputs["w1"])
    b1 = np.asarray(inputs["b1"])
    w2 = np.asarray(inputs["w2"])
    b2 = np.asarray(inputs["b2"])
    wm = np.asarray(inputs["wm"])
    bm = np.asarray(inputs["bm"])
    wc = np.asarray(inputs["wc"])
    bc = np.asarray(inputs["bc"])
    w3 = np.asarray(inputs["w3"])
    b3 = np.asarray(inputs["b3"])

    f8np = ml_dtypes.float8_e4m3
    # pre-tiled conv1 input (fp8, 2x scaled) with dy, dx AND the stride-2
    # x-subsample baked in:
    # xb3[core][q, r, (dy,ci,dx), c, y, x'] = 2*x[core*8+2c+r, ci,
    #                                            2*(16q+y)+dy-1, 2x'+dx-1]
    NQH = H1 // YC_HOST
    xb27 = np.zeros((NCORES, NQH, 2, 27, 4, YC_HOST, W1), dtype=np.float32)
    rows = 2 * (np.arange(NQH * YC_HOST).reshape(NQH, YC_HOST))[None, None, :, :] \
        + np.arange(3).reshape(1, 3, 1, 1) - 1          # [1, dy, q, y]
    valid = (rows >= 0) & (rows < H)
    rowsc = np.clip(rows, 0, H - 1)
    xcols = 2 * np.arange(W1)[None, :] + np.arange(3)[:, None] - 1   # [dx, x']
    xvalid = ((xcols >= 0) & (xcols < W)).astype(np.float32)
    xc = np.clip(xcols, 0, W - 1)
    for r in range(2):
        for c in range(4):
            img = 2.0 * x[2 * c + r::BL, :, :, :]       # [NCORES, 3, H, W]
            g = img[:, :, rowsc[0], :]                  # [NCORES, ci, dy, q, y, W]
            g = g * valid[0][None, None, :, :, :, None]
            g2 = g[..., xc] * xvalid[None, None, None, None, None, :, :]
            # [N, ci, dy, q, y, dx, x'] -> [N, q, (dy,ci,dx), y, x']
            xb27[:, :, r, :, c, :, :] = (
                g2.transpose(0, 3, 2, 1, 5, 4, 6)
                  .reshape(NCORES, NQH, 27, YC_HOST, W1))
    # block-diagonal packing: 4 images stacked in K (rows 27c+t)
    xb3f = np.zeros((NCORES, NQH, 108, 2, YC_HOST, W1), np.float32)
    for c in range(4):
        xb3f[:, :, 27 * c:27 * c + 27, :, :, :] = (
            xb27[:, :, :, :, c, :, :].transpose(0, 1, 3, 2, 4, 5))
    xb3 = xb3f.astype(f8np)
    w1t27 = 8.0 * w1.transpose(2, 1, 3, 0).reshape(27, 32)
    w1blk = np.zeros((108, 128), np.float32)
    for c in range(4):
        w1blk[27 * c:27 * c + 27, 32 * c:32 * c + 32] = w1t27
    w1t = w1blk.astype(f8np)
    # block-diagonal conv2 weights: rows 32h+ci, cols 64h+co; duplicated to
    # both partition halves (rows 64-127 = rows 0-63)
    w2tr = 8.0 * w2.transpose(1, 2, 3, 0)      # [ci, dy, dx, co]
    w2b = np.zeros((64, 3, 3, 128), np.float32)
    for h in range(2):
        w2b[32 * h:32 * h + 32, :, :, 64 * h:64 * h + 64] = w2tr
    w2t = np.ascontiguousarray(
        np.concatenate([w2b, w2b], axis=0)).astype(f8np)
    b1r = np.ascontiguousarray(16.0 * b1.reshape(32, 1)).astype(np.float32)
    b2r = np.ascontiguousarray(128.0 * b2.reshape(64, 1)).astype(np.float32)

    w3np = ml_dtypes.float8_e4m3

    # UT pair index arrays, j-major: p -> (jj[p] <= kk[p])
    jj = np.concatenate([np.full(64 - j, j, np.int64) for j in range(64)])
    kk = np.concatenate([np.arange(j, 64) for j in range(64)])

    # symmetrize w3 over (i,j,k): Wsym = sum over all 6 axis permutations,
    # done in o-blocks for cache locality
    W4 = w3.reshape(8 * OPC, 64, 64, 64)
    Wsym = np.empty_like(W4)
    for o0 in range(0, 8 * OPC, 16):
        blk = W4[o0:o0 + 16]
        A = blk + blk.transpose(0, 1, 3, 2)
        Wsym[o0:o0 + 16] = A + A.transpose(0, 2, 1, 3) + A.transpose(0, 3, 2, 1)

    ocols = 4 * np.arange(64)
    in_maps = []
    for c in range(NCORES):
        osl = slice(OPC * c, OPC * (c + 1))
        Wc = Wsym[osl]                       # [128, 64, 64, 64]
        blobs = np.zeros((NCH, 128, 512), np.float32)   # [chunk, p, x]
        ci = 0
        for g in range(NG):
            P = np.arange(128 * KCS[g], NP)
            J = jj[P]
            Kq = kk[P]
            blk = np.zeros((len(P), 512), np.float32)
            for il in range(GRP):
                i = GRP * g + il
                mask = (J >= i).astype(np.float32)
                d = np.where((i == J) & (J == Kq), 6.0,
                             np.where((i == J) | (J == Kq), 2.0, 1.0))
                vals = Wc[:, i, J, Kq] * (mask / d)   # [128, len(P)]
                blk[:, ocols + il] = vals[0:64].T
                blk[:, 256 + ocols + il] = vals[64:128].T
            r0 = 0
            for kc in range(KCS[g], NPC):
                rows = min(128, NP - 128 * kc)
                blobs[ci, 0:rows, :] = blk[r0:r0 + rows, :]
                r0 += rows
                ci += 1
        w3gc = np.ascontiguousarray(
            blobs.transpose(1, 0, 2)).astype(w3np)     # (128, NCH, 512)

        # wc folded over its symmetric (j,k) -> UT rows, zero-padded to NPPAD;
        # scaled by 1/CSCALE to compensate the fp8 covT scaling
        wcv = wc[osl].reshape(OPC, 64, 64).astype(np.float32)
        wcf = (wcv[:, jj, kk] + (jj < kk).astype(np.float32) * wcv[:, kk, jj])
        wcfp = np.zeros((NPPAD, OPC), np.float16)
        wcfp[0:NP, :] = (wcf.T / CSCALE).astype(np.float16)

        in_maps.append({
            "xb3": np.ascontiguousarray(xb3[c]),
            "w1t": w1t,
            "b1": b1r,
            "w2t": w2t,
            "b2": b2r,
            "wmt": np.ascontiguousarray(wm[osl].T).astype(np.float32),
            "wct": wcfp,
            "w3g": w3gc,
            "bias3": np.stack([bm[osl], bc[osl], b3[osl]]).astype(np.float32),
        })

    return in_maps


def kernel(**inputs):
    in_maps = _prepare_in_maps(inputs)
    from concourse.bass_utils import run_bass_kernel_spmd

    res = run_bass_kernel_spmd(_get_nc(), in_maps, core_ids=list(range(NCORES)))
    return np.concatenate([res.results[c]["out"] for c in range(NCORES)], axis=1)


if __name__ == "__main__":
    nc = _build()
    print("built OK; instructions:",
          sum(len(bb.instructions) for f in nc.m.functions for bb in f.blocks))
    if "compile" in sys.argv:
        import tempfile
        from concourse.bass_utils import compile_bass_kernel
        d = tempfile.mkdtemp()
        print("compiling in", d)
        print("NEFF:", compile_bass_kernel(nc, d))



# revision 61
# speedup vs baseline: 1.4759x; 1.4759x over previous
"""Trainium2 Bass kernel for nn_NoiseProjector.

Strategy (8 NeuronCores):
- Data-parallel conv trunk: each core runs conv1+conv2+GAP on 8 of 64 images.
- Tiny AllGather of the pooled features (8x64 -> 64x64 per core).
- Tensor-parallel FC stage: weights column-sharded over output_dim (128
  outputs per core).
- The third-order contraction  sum_{ijk} fc_i fc_j fc_k W3[o,ijk]  depends
  only on the symmetrization of W3 over (i,j,k).  Host folds all 6
  permutations into a canonical i<=j<=k tensor W''[o,i,(j<=k)] (zero for
  j<i), so the device streams only the block-triangular part:
    T1[b,o,i] = sum_{p=(j<=k), j>=i} covUT[b,p] * W''[o,i,p]
    third[b,o] = sum_i fc[b,i] * T1[b,o,i]
  where covUT[b, p] = fc_j fc_k over the 2080 upper-triangle pairs.
  i is processed in 16 groups of 4 with K-ranges rounded down to the global
  128-chunk grid of covUT^T; weight blob is 13184 rows x 512 cols fp8e4m3
  = 6.75 MB/core (vs 33.5 MB unfolded fp8), with 512B DMA lines.
- cov/mean fc: wc is folded the same way over its symmetric (j,k) -> 2080
  rows f16 (scaled 1/CSCALE to compensate the fp8 covT scaling); wm dense.
- The w3 stream is prefetched into a persistent SBUF buffer by 8 segment
  DMAs on the gpsimd DGE ring at kernel start, hiding it under the conv.
- The w3 matmuls run fp8 DoubleRow (2 K-chunks per pass); covT is fp8
  scaled by CSCALE=512 so values sit in e4m3's normal range.
- Conv trunk in fp8 (x*2, w1*8, w2*8, h1=16*a stored fp8; GAP compensates
  by 1/128): conv1 bakes dy, dx AND the stride-2 subsample into the host
  pre-tiled input (27 K-rows, no tap re-streaming) and packs 4 images
  block-diagonally into one K=108 x M=128 full-width matmul; conv2 packs
  2 images per matmul (K=64, M=128, weights duplicated per partition
  quadrant).  Block-diagonal packing needs no PE tile concurrency: the
  streamed column count itself drops 4x/2x.
- Host concatenates the 8 per-core (64,128) outputs into (64,1024).
"""

import sys

sys.path.insert(0, "/opt/trn_rl_repo")

import numpy as np
import ml_dtypes

B = 64          # global batch
BL = 8          # images per core
NCORES = 8
OPC = 128       # outputs per core
FEAT = 64
JK = 4096       # FEAT*FEAT contraction
NP = 2080       # upper-triangle (j<=k) pairs
NPC = 17        # ceil(NP/128) 128-chunks of the UT pair space
NPPAD = NPC * 128            # 2176, zero-padded pair rows for wc
OFFJ = [64 * j - j * (j - 1) // 2 for j in range(65)]   # UT row offset per j
GRP = 4         # i's per group
NG = 16         # i groups
KCS = [OFFJ[GRP * g] // 128 for g in range(NG)]  # start chunk per group
TOTROWS = sum(128 * (16 - k) + 32 for k in KCS)  # 13184 blob rows
NCH = sum(NPC - k for k in KCS)                  # 115 weight chunks
CSCALE = 512.0  # covUT scaled by this before fp8 cast (wc/stage2 compensate)
H, W = 224, 224
H1, W1 = 112, 112   # conv1 out
H2, W2 = 56, 56     # conv2 out
YC_HOST = 16        # conv1 y-rows per chunk (host pre-tiled layout)
GAP = 1.0 / (H2 * W2)



def _split_multiwait_json(raw):
    """This walrus build accepts only ONE sync wait per instruction.  Split any
    multi-wait instruction into single-wait EventSemaphore ops ahead of it (the
    engine is in-order, so chained waits are equivalent)."""
    import json

    j = json.loads(raw)
    n_split = 0
    for f in j["functions"]:
        for bb in f["blocks"]:
            insts = bb.get("instructions")
            if not insts:
                continue
            out = []
            changed = False
            for ins in insts:
                si = ins.get("sync_info")
                waits = si.get("on_wait") if si else None
                if waits and len(waits) > 1:
                    changed = True
                    keep = None
                    for w in waits:
                        if w.get("wait_reg") is not None:
                            keep = w
                    if keep is None:
                        keep = waits[-1]
                    rest = [w for w in waits if w is not keep]
                    for k, w in enumerate(rest):
                        n_split += 1
                        out.append({
                            "engine": ins["engine"], "ins": [], "outs": [],
                            "name": f"{ins['name']}-sw{k}",
                            "opcode": "EventSemaphore",
                            "sync_info": {"on_update": [], "on_wait": [w]},
                        })
                    si["on_wait"] = [keep]
                out.append(ins)
            if changed:
                bb["instructions"] = out
    return json.dumps(j).encode(), n_split


def _build(reps=1, trivial=False, w3_fp8=None, conv_reps=1, conv1_only=False,
           conv2_seq=False, coll_reps=1):
    YC = YC_HOST
    NQ = H1 // YC
    import concourse.bass as bass
    import concourse.mybir as mybir
    import concourse.tile as tile
    from concourse.masks import make_identity

    F32, F16, BF16 = mybir.dt.float32, mybir.dt.float16, mybir.dt.bfloat16
    AF = mybir.ActivationFunctionType
    ALU = mybir.AluOpType
    AX = mybir.AxisListType



    nc = bass.Bass("TRN2", target_bir_lowering=False, num_devices=NCORES)

    F8 = mybir.dt.float8e4
    DRm = mybir.MatmulPerfMode.DoubleRow
    xb3 = nc.dram_tensor(
        "xb3", (H1 // YC_HOST, 108, 2, YC_HOST, W1), F8,
        kind="ExternalInput").ap()
    w1t = nc.dram_tensor("w1t", (108, 128), F8, kind="ExternalInput").ap()
    b1 = nc.dram_tensor("b1", (32, 1), F32, kind="ExternalInput").ap()
    w2t = nc.dram_tensor("w2t", (128, 3, 3, 128), F8, kind="ExternalInput").ap()
    b2 = nc.dram_tensor("b2", (64, 1), F32, kind="ExternalInput").ap()
    wmt = nc.dram_tensor("wmt", (64, OPC), F32, kind="ExternalInput").ap()
    w3dt = F8
    w3g = nc.dram_tensor("w3g", (128, NCH, 512), w3dt, kind="ExternalInput").ap()
    wct = nc.dram_tensor("wct", (NPPAD, OPC), F16, kind="ExternalInput").ap()
    bias3 = nc.dram_tensor("bias3", (3, OPC), F32, kind="ExternalInput").ap()
    out = nc.dram_tensor("out", (B, OPC), F32, kind="ExternalOutput").ap()
    feat_loc = nc.dram_tensor("feat_loc", (BL, FEAT), F32).ap()
    feat_all = nc.dram_tensor("feat_all", (B, FEAT), F32, addr_space="Shared").ap()

    if trivial:
        with tile.TileContext(nc) as tc:
            with tc.tile_pool(name="tp", bufs=1) as tp:
                z = tp.tile([B, OPC], F32)
                nc.vector.memset(z[:], 0.0)
                nc.sync.dma_start(out[:], z[:])
        nc.finalize()
        fixed, _ = _split_multiwait_json(nc.to_json_bytes())
        nc.to_json_bytes = lambda: fixed
        return nc

    with tile.TileContext(nc) as tc:
        with (
            tc.tile_pool(name="consts", bufs=1) as consts,
            tc.tile_pool(name="fcsingle", bufs=1) as fcsingle,
            tc.tile_pool(name="fcwork", bufs=2) as fcwork,
        ):
            # ---- constants ----
            # block-diagonal conv1 weights: rows 27c+(dy,ci,dx), cols 32c+co
            w1sb = consts.tile([108, 128], F8)
            nc.sync.dma_start(w1sb[:], w1t[:])
            # block-diagonal conv2 weights (2-img blocks), duplicated to both
            # partition halves (stationary must sit in its own row quadrant)
            w2sb = consts.tile([128, 3, 3, 128], F8)
            nc.sync.dma_start(w2sb[:], w2t[:])
            bias1 = consts.tile([128, 1], F32)          # b1[cout] at 32c+cout
            nc.sync.dma_start(
                bias1[:],
                bass.AP(tensor=b1.tensor, offset=0, ap=[[0, 4], [1, 32], [1, 1]]),
            )
            bias2 = consts.tile([128, 1], F32)          # 128x b2 at 64h+co
            nc.sync.dma_start(
                bias2[:],
                bass.AP(tensor=b2.tensor, offset=0, ap=[[0, 2], [1, 64], [1, 1]]),
            )
            wmsb = consts.tile([64, OPC], F32)
            nc.sync.dma_start(wmsb[:], wmt[:])
            wcsb = consts.tile([128, NPC, OPC], F16)    # [p, kc, m] of folded wct
            nc.sync.dma_start(
                wcsb[:], wct[:].rearrange("(kc p) m -> p kc m", p=128)
            )
            bias3sb = consts.tile([64, 3, OPC], F32)
            nc.sync.dma_start(
                bias3sb[:],
                bass.AP(tensor=bias3.tensor, offset=0,
                        ap=[[0, 64], [OPC, 3], [1, OPC]]),
            )
            bsum = consts.tile([64, OPC], F32)
            nc.vector.tensor_reduce(bsum[:], bias3sb[:].transpose([0, 2, 1]),
                                    AX.X, op=ALU.add)
            ident = consts.tile([64, 64], F32)
            make_identity(nc, ident[:])

            # persistent w3 buffer in 8 group-aligned segments; prefetch DMAs
            # issued on the gpsimd queue (own DGE ring) right at kernel start
            # so the whole 7.5MB stream overlaps the conv trunk.  Segmenting
            # lets measurement reps overlap the re-stream with compute.
            GCNT = [NPC - k for k in KCS]            # chunks per group
            SEGG = [[0], [1], [2], [3], [4, 5], [6, 7], [8, 9, 10],
                    [11, 12, 13, 14, 15]]
            w3segs = []
            SEGOF = {}
            for si, gl in enumerate(SEGG):
                n = sum(GCNT[g] for g in gl)
                w3segs.append(consts.tile([128, n, 512], w3dt,
                                          name=f"w3seg{si}"))
                base = sum(GCNT[g] for g in range(gl[0]))
                for lo in range(n):
                    SEGOF[base + lo] = (si, lo)

            def _w3_load():
                c0 = 0
                for si, gl in enumerate(SEGG):
                    n = sum(GCNT[g] for g in gl)
                    nc.gpsimd.dma_start(w3segs[si][:], w3g[:, c0:c0 + n, :])
                    c0 += n

            def _w3slot(ci, nt):
                si, lo = SEGOF[ci]
                if nt == 2:
                    return w3segs[si][:, lo:lo + 2, :]
                return w3segs[si][:, lo, :]

            _w3_load()
            zeros = consts.tile([128, 448], BF16)
            nc.vector.memset(zeros[:], 0.0)
            featparts = consts.tile([128, 2, 2, 7], F32)   # [64h+co, p2, r, sc]
            if conv1_only:
                nc.vector.memset(featparts[:], 0.0)

            # =============== conv trunk (8 local images) ===============
            # conv1: K=(dy,ci)=9 at row groups {0,32}; 4 images per col group.
            # image assignment: img = 2*c + r  (c: col group, r: row group/bank)
            with (
                tc.tile_pool(name="conv", bufs=2) as conv,
                tc.tile_pool(name="h1p", bufs=1) as h1p,
            ):
                # h1 stored fp8, 16x scaled (x*2 * w1*8)
                h1 = h1p.tile([128, 2, H1 + 2, W1 + 2], F8)  # [(32c)+co, r, y+1, x+1]
                nc.vector.memset(h1[:, :, 0:1, :], 0.0)        # top pad row
                nc.vector.memset(h1[:, :, :, 0:1], 0.0)        # left pad col

                def _win(slc, dims):
                    # overlapping-window AP: keep the partition entry of a
                    # sliced AP, replace the free dims
                    return bass.AP(tensor=slc.tensor, offset=slc.offset,
                                   ap=[list(slc.ap[0])] + dims)

                for _crep in range(conv_reps):
                    cpsum_cm = tc.tile_pool(name="cpsum", bufs=3, space="PSUM")
                    cpsum = cpsum_cm.__enter__()
                    for q in range(NQ):
                        a1 = conv.tile([108, 2, YC, W1], F8, tag="a1")
                        nc.sync.dma_start(a1[:], xb3[q])
                        for s in range(YC // 4):
                            ps1 = cpsum.tile([128, 2, 512], F32, tag="cpsum")
                            for rg in range(2):
                                rhs = a1[:, rg, 4 * s:4 * s + 4, :]
                                nc.tensor.matmul(
                                    ps1[:, rg, 0:448],
                                    w1sb[:],
                                    rhs,
                                    start=True, stop=True,
                                )
                            ybase = 1 + q * YC + 4 * s
                            for r in range(2):
                                src = ps1[:, r, 0:448].rearrange("p (y x) -> p y x", y=4)
                                dst = h1[:, r, ybase:ybase + 4, 1:113]
                                if r == 0:
                                    nc.scalar.activation(dst, src, AF.Relu,
                                                         bias=bias1[:], scale=1.0)
                                else:
                                    nc.vector.scalar_tensor_tensor(
                                        dst, src, bias1[:],
                                        zeros[:].rearrange("p (y x) -> p y x", y=4),
                                        op0=ALU.add, op1=ALU.max,
                                    )

                    cpsum_cm.__exit__(None, None, None)

                    if conv1_only:
                        continue
                    # conv2: K=ci=32 at row groups {0,32,64,96} (4 images per
                    # r), M=64; fp8 DoubleRow pairs (dy0,dy1)x dx, (dy2:dx0,dx1),
                    # plus plain (dy2,dx2); 2 y-halves of 224 cols each.
                    with tc.tile_pool(name="c2psum", bufs=2, space="PSUM") as c2psum:
                        trash = consts.tile([128, 448], BF16)
                        trash2 = consts.tile([128, 448], BF16)
                        for r in range(2):
                            for sc in range(7):
                                ps2 = c2psum.tile([128, 2, 512], F32, tag="c2psum")
                                for dy in range(3):
                                    for dx in range(3):
                                        for p2 in range(2):
                                            rhs = h1[64 * p2:64 * p2 + 64, r,
                                                     16 * sc + dy:16 * sc + dy + 16:2,
                                                     dx:dx + 2 * W2:2]
                                            nc.tensor.matmul(
                                                ps2[:, p2, 0:448],
                                                w2sb[64 * p2:64 * p2 + 64, dy, dx, :],
                                                rhs,
                                                start=(dy == 0 and dx == 0),
                                                stop=(dy == 2 and dx == 2),
                                                tile_position=(64 * p2, 0),
                                                skip_group_check=True,
                                            )
                                for p2 in range(2):
                                    for h in range(2):
                                        src = ps2[64 * h:64 * h + 64, p2, 0:448]
                                        fslot = featparts[64 * h:64 * h + 64,
                                                          p2, r, sc:sc + 1]
                                        if (p2 + h) % 2 == 0:
                                            nc.scalar.activation(
                                                trash[64 * h:64 * h + 64, :], src,
                                                AF.Relu,
                                                bias=bias2[64 * h:64 * h + 64],
                                                scale=1.0, accum_out=fslot,
                                            )
                                        else:
                                            nc.vector.scalar_tensor_tensor(
                                                trash2[64 * h:64 * h + 64, :], src,
                                                bias2[64 * h:64 * h + 64],
                                                zeros[64 * h:64 * h + 64, :],
                                                op0=ALU.add, op1=ALU.max,
                                                accum_out=fslot,
                                            )

            # featparts[64h+co, p2, r, sc] holds img 4*p2+2*h+r; move the h=1
            # half down to partitions 0-63 and reduce over sc into featTl
            fpfix = fcsingle.tile([64, 2, 2, 7], F32, tag="fpfix")
            nc.sync.dma_start(fpfix[:], featparts[64:128, :, :, :])
            featTl = fcsingle.tile([64, BL], F32, tag="featTl")
            for p2 in range(2):
                for r in range(2):
                    nc.vector.tensor_reduce(
                        featTl[:, 4 * p2 + r:4 * p2 + r + 1],
                        featparts[0:64, p2, r, :], AX.X, op=ALU.add)
                    nc.vector.tensor_reduce(
                        featTl[:, 4 * p2 + 2 + r:4 * p2 + 3 + r],
                        fpfix[:, p2, r, :], AX.X, op=ALU.add)
            nc.vector.tensor_scalar_mul(featTl[:], featTl[:], GAP / 128.0)
            nc.sync.dma_start(feat_loc[:].transpose([1, 0]), featTl[:])

            for _cl in range(coll_reps):
                nc.gpsimd.collective_compute(
                    "AllGather", ALU.bypass,
                    replica_groups=[list(range(NCORES))],
                    ins=[feat_loc[:]], outs=[feat_all[:]],
                )

            for _rep in range(reps):
                # =============== fc prep ===============
                covT = fcsingle.tile([128, NPC, 64], F8, tag="covT")
                featT = fcsingle.tile([64, 64], F32, tag="featT")
                thirdparts = fcsingle.tile([64, OPC, NG], F32, tag="thirdparts")
                feat = fcsingle.tile([64, 64], F32, tag="feat")
                nc.sync.dma_start(feat[:], feat_all[:])
                mean = fcsingle.tile([64, 1], F32, tag="mean")
                nc.vector.tensor_reduce(mean[:], feat[:], AX.X, op=ALU.add)
                nc.vector.tensor_scalar_mul(mean[:], mean[:], 1.0 / FEAT)
                fc = fcsingle.tile([64, 64], F32, tag="fc")
                nc.vector.tensor_scalar_sub(fc[:], feat[:], mean[:])
                fcS = fcsingle.tile([64, 64], F32, tag="fcS")
                nc.vector.tensor_scalar_mul(fcS[:], fc[:], CSCALE)
                fcI = fcsingle.tile([64, 64], F32, tag="fcI")
                nc.vector.tensor_scalar_mul(fcI[:], fc[:], 1.0 / CSCALE)
                # covUT[b, p] = CSCALE * fc_j * fc_k over UT pairs (j<=k),
                # j-major; construction split across vector and scalar engines
                covUT = fcsingle.tile([64, NP], F32, tag="covUT")
                for j in range(64):
                    if j % 2 == 0:
                        nc.vector.tensor_scalar_mul(
                            covUT[:, OFFJ[j]:OFFJ[j + 1]], fc[:, j:64],
                            fcS[:, j:j + 1])
                    else:
                        nc.scalar.mul(
                            covUT[:, OFFJ[j]:OFFJ[j + 1]], fc[:, j:64],
                            fcS[:, j:j + 1])

                with tc.tile_pool(name="fpsum", bufs=2, space="PSUM") as fpsum, \
                     tc.tile_pool(name="wpsum", bufs=1, space="PSUM") as wpsum:
                    # transposes: covUT -> covT chunks; feat -> featT
                    for kc in range(NPC):
                        cw = min(128, NP - 128 * kc)
                        pT = fpsum.tile([128, 64], F32, tag="pT")
                        nc.tensor.transpose(
                            pT[0:cw, :], covUT[:, 128 * kc:128 * kc + cw], ident[:])
                        if kc % 2 == 0:
                            nc.vector.tensor_copy(covT[0:cw, kc, :], pT[0:cw, :])
                        else:
                            nc.scalar.copy(covT[0:cw, kc, :], pT[0:cw, :])
                    pT2 = fpsum.tile([128, 64], F32, tag="pT")
                    nc.tensor.transpose(pT2[0:64, :], feat[:], ident[:])
                    nc.vector.tensor_copy(featT[:], pT2[0:64, :])

                    # wc + wm accumulation -> psum_wc [64, 128]
                    pwc = wpsum.tile([64, OPC], F32)
                    for kc in range(NPC):
                        cw = min(128, NP - 128 * kc)
                        nc.tensor.matmul(pwc[:], covT[0:cw, kc, :], wcsb[0:cw, kc, :],
                                         start=(kc == 0), stop=False)
                    nc.tensor.matmul(pwc[:], featT[:], wmsb[:], start=False, stop=True)

                    # ---- w3 stream: 16 i-groups, block-triangular K ranges ----
                    if _rep > 0:
                        _w3_load()   # measurement reps re-stream the weights
                    DR = mybir.MatmulPerfMode.DoubleRow
                    ci = 0
                    for g in range(NG):
                        pba = fpsum.tile([64, 256], F32, tag="pba")
                        pbb = fpsum.tile([64, 256], F32, tag="pbb")
                        kcl = list(range(KCS[g], NPC))
                        # pair full chunks for DoubleRow fp8 (2 k-tiles/pass),
                        # then odd full chunk, then the 32-row tail (kc==16)
                        items = []
                        idx = 0
                        while idx + 2 < len(kcl):
                            items.append((2, kcl[idx], ci))
                            idx += 2
                            ci += 2
                        while idx < len(kcl):
                            items.append((1, kcl[idx], ci))
                            idx += 1
                            ci += 1
                        for it, (nt, kc, c0) in enumerate(items):
                            st, sp = (it == 0), (it == len(items) - 1)
                            rows = min(128, NP - 128 * kc)
                            wsl = _w3slot(c0, nt)
                            if nt == 2:
                                lhs = covT[:, kc:kc + 2, :]
                                r0 = wsl[:, :, 0:256]
                                r1 = wsl[:, :, 256:512]
                                pm = DR
                            else:
                                lhs = covT[0:rows, kc, :]
                                r0 = wsl[0:rows, 0:256]
                                r1 = wsl[0:rows, 256:512]
                                pm = None
                            nc.tensor.matmul(pba[:], lhs, r0,
                                             start=st, stop=sp, perf_mode=pm,
                                             tile_position=(0, 0),
                                             skip_group_check=True)
                            nc.tensor.matmul(pbb[:], lhs, r1,
                                             start=st, stop=sp, perf_mode=pm,
                                             tile_position=(0, 0),
                                             skip_group_check=True)
                        # stage 2: third[b, o] += sum_il fc[b, 4g+il] * T1[b, o, il]
                        # a-half: vector mul+reduce; b-half: scalar copies psum
                        # out, gpsimd multiplies, vector reduces (f16 tmps)
                        fcg = fcI[:, GRP * g:GRP * (g + 1)]
                        tmpa = fcwork.tile([64, 256], F16, tag="tmpa")
                        nc.vector.tensor_mul(
                            tmpa[:].rearrange("p (o i) -> p o i", o=64),
                            pba[:].rearrange("p (o i) -> p o i", o=64),
                            fcg.unsqueeze(1).broadcast_to([64, 64, GRP]),
                        )
                        nc.vector.tensor_reduce(
                            thirdparts[:, 0:64, g],
                            tmpa[:].rearrange("p (o i) -> p o i", o=64),
                            AX.X, op=ALU.add,
                        )
                        tmpb = fcwork.tile([64, 256], F16, tag="tmpb")
                        nc.vector.tensor_mul(
                            tmpb[:].rearrange("p (o i) -> p o i", o=64),
                            pbb[:].rearrange("p (o i) -> p o i", o=64),
                            fcg.unsqueeze(1).broadcast_to([64, 64, GRP]),
                        )
                        nc.vector.tensor_reduce(
                            thirdparts[:, 64:128, g],
                            tmpb[:].rearrange("p (o i) -> p o i", o=64),
                            AX.X, op=ALU.add,
                        )

                    # ---- final assembly: out = third + (cov_feat+mean_feat) + biases
                    acc3 = fcsingle.tile([64, OPC], F32, tag="acc3")
                    nc.vector.tensor_reduce(acc3[:], thirdparts[:], AX.X, op=ALU.add)
                    acc = fcsingle.tile([64, OPC], F32, tag="acc")
                    nc.vector.tensor_add(acc[:], acc3[:], pwc[:])
                    nc.vector.tensor_add(acc[:], acc[:], bsum[:])
                    nc.sync.dma_start(out[:], acc[:])

    nc.finalize()
    fixed, n_split = _split_multiwait_json(nc.to_json_bytes())
    nc.to_json_bytes = lambda: fixed
    return nc


_NC_CACHE = None


def _get_nc():
    global _NC_CACHE
    if _NC_CACHE is None:
        _NC_CACHE = _build()
    return _NC_CACHE


def _prepare_in_maps(inputs):
    x = np.asarray(inputs["x"])
    w1 = np.asarray(inputs["w1"])
    b1 = np.asarray(inputs["b1"])
    w2 = np.asarray(inputs["w2"])
    b2 = np.asarray(inputs["b2"])
    wm = np.asarray(inputs["wm"])
    bm = np.asarray(inputs["bm"])
    wc = np.asarray(inputs["wc"])
    bc = np.asarray(inputs["bc"])
    w3 = np.asarray(inputs["w3"])
    b3 = np.asarray(inputs["b3"])

    f8np = ml_dtypes.float8_e4m3
    # pre-tiled conv1 input (fp8, 2x scaled) with dy, dx AND the stride-2
    # x-subsample baked in:
    # xb3[core][q, r, (dy,ci,dx), c, y, x'] = 2*x[core*8+2c+r, ci,
    #                                            2*(16q+y)+dy-1, 2x'+dx-1]
    NQH = H1 // YC_HOST
    xb27 = np.zeros((NCORES, NQH, 2, 27, 4, YC_HOST, W1), dtype=np.float32)
    rows = 2 * (np.arange(NQH * YC_HOST).reshape(NQH, YC_HOST))[None, None, :, :] \
        + np.arange(3).reshape(1, 3, 1, 1) - 1          # [1, dy, q, y]
    valid = (rows >= 0) & (rows < H)
    rowsc = np.clip(rows, 0, H - 1)
    xcols = 2 * np.arange(W1)[None, :] + np.arange(3)[:, None] - 1   # [dx, x']
    xvalid = ((xcols >= 0) & (xcols < W)).astype(np.float32)
    xc = np.clip(xcols, 0, W - 1)
    for r in range(2):
        for c in range(4):
            img = 2.0 * x[2 * c + r::BL, :, :, :]       # [NCORES, 3, H, W]
            g = img[:, :, rowsc[0], :]                  # [NCORES, ci, dy, q, y, W]
            g = g * valid[0][None, None, :, :, :, None]
            g2 = g[..., xc] * xvalid[None, None, None, None, None, :, :]
            # [N, ci, dy, q, y, dx, x'] -> [N, q, (dy,ci,dx), y, x']
            xb27[:, :, r, :, c, :, :] = (
                g2.transpose(0, 3, 2, 1, 5, 4, 6)
                  .reshape(NCORES, NQH, 27, YC_HOST, W1))
    # block-diagonal packing: 4 images stacked in K (rows 27c+t)
    xb3f = np.zeros((NCORES, NQH, 108, 2, YC_HOST, W1), np.float32)
    for c in range(4):
        xb3f[:, :, 27 * c:27 * c + 27, :, :, :] = (
            xb27[:, :, :, :, c, :, :].transpose(0, 1, 3, 2, 4, 5))
    xb3 = xb3f.astype(f8np)
    w1t27 = 8.0 * w1.transpose(2, 1, 3, 0).reshape(27, 32)
    w1blk = np.zeros((108, 128), np.float32)
    for c in range(4):
        w1blk[27 * c:27 * c + 27, 32 * c:32 * c + 32] = w1t27
    w1t = w1blk.astype(f8np)
    # block-diagonal conv2 weights: rows 32h+ci, cols 64h+co; duplicated to
    # both partition halves (rows 64-127 = rows 0-63)
    w2tr = 8.0 * w2.transpose(1, 2, 3, 0)      # [ci, dy, dx, co]
    w2b = np.zeros((64, 3, 3, 128), np.float32)
    for h in range(2):
        w2b[32 * h:32 * h + 32, :, :, 64 * h:64 * h + 64] = w2tr
    w2t = np.ascontiguousarray(
        np.concatenate([w2b, w2b], axis=0)).astype(f8np)
    b1r = np.ascontiguousarray(16.0 * b1.reshape(32, 1)).astype(np.float32)
    b2r = np.ascontiguousarray(128.0 * b2.reshape(64, 1)).astype(np.float32)

    w3np = ml_dtypes.float8_e4m3

    # UT pair index arrays, j-major: p -> (jj[p] <= kk[p])
    jj = np.concatenate([np.full(64 - j, j, np.int64) for j in range(64)])
    kk = np.concatenate([np.arange(j, 64) for j in range(64)])

    # symmetrize w3 over (i,j,k): Wsym = sum over all 6 axis permutations,
    # done in o-blocks for cache locality
    W4 = w3.reshape(8 * OPC, 64, 64, 64)
    Wsym = np.empty_like(W4)
    for o0 in range(0, 8 * OPC, 16):
        blk = W4[o0:o0 + 16]
        A = blk + blk.transpose(0, 1, 3, 2)
        Wsym[o0:o0 + 16] = A + A.transpose(0, 2, 1, 3) + A.transpose(0, 3, 2, 1)

    ocols = 4 * np.arange(64)
    in_maps = []
    for c in range(NCORES):
        osl = slice(OPC * c, OPC * (c + 1))
        Wc = Wsym[osl]                       # [128, 64, 64, 64]
        blobs = np.zeros((NCH, 128, 512), np.float32)   # [chunk, p, x]
        ci = 0
        for g in range(NG):
            P = np.arange(128 * KCS[g], NP)
            J = jj[P]
            Kq = kk[P]
            blk = np.zeros((len(P), 512), np.float32)
            for il in range(GRP):
                i = GRP * g + il
                mask = (J >= i).astype(np.float32)
                d = np.where((i == J) & (J == Kq), 6.0,
                             np.where((i == J) | (J == Kq), 2.0, 1.0))
                vals = Wc[:, i, J, Kq] * (mask / d)   # [128, len(P)]
                blk[:, ocols + il] = vals[0:64].T
                blk[:, 256 + ocols + il] = vals[64:128].T
            r0 = 0
            for kc in range(KCS[g], NPC):
                rows = min(128, NP - 128 * kc)
                blobs[ci, 0:rows, :] = blk[r0:r0 + rows, :]
                r0 += rows
                ci += 1
        w3gc = np.ascontiguousarray(
            blobs.transpose(1, 0, 2)).astype(w3np)     # (128, NCH, 512)

        # wc folded over its symmetric (j,k) -> UT rows, zero-padded to NPPAD;
        # scaled by 1/CSCALE to compensate the fp8 covT scaling
        wcv = wc[osl].reshape(OPC, 64, 64).astype(np.float32)
        wcf = (wcv[:, jj, kk] + (jj < kk).astype(np.float32) * wcv[:, kk, jj])
        wcfp = np.zeros((NPPAD, OPC), np.float16)
        wcfp[0:NP, :] = (wcf.T / CSCALE).astype(np.float16)

        in_maps.append({
            "xb3": np.ascontiguousarray(xb3[c]),
            "w1t": w1t,
            "b1": b1r,
            "w2t": w2t,
            "b2": b2r,
            "wmt": np.ascontiguousarray(wm[osl].T).astype(np.float32),
            "wct": wcfp,
            "w3g": w3gc,
            "bias3": np.stack([bm[osl], bc[osl], b3[osl]]).astype(np.float32),
        })

    return in_maps


def kernel(**inputs):
    in_maps = _prepare_in_maps(inputs)
    from concourse.bass_utils import run_bass_kernel_spmd

    res = run_bass_kernel_spmd(_get_nc(), in_maps, core_ids=list(range(NCORES)))
    return np.concatenate([res.results[c]["out"] for c in range(NCORES)], axis=1)


if __name__ == "__main__":
    nc = _build()
    print("built OK; instructions:",
          sum(len(bb.instructions) for f in nc.m.functions for bb in f.blocks))
    if "compile" in sys.argv:
        import tempfile
        from concourse.bass_utils import compile_bass_kernel
        d = tempfile.mkdtemp()
        print("compiling in", d)
        print("NEFF:", compile_bass_kernel(nc, d))

